# revision 1
# baseline (speedup 1.0000x reference)
"""DrugGNN segment-mean + linear embed, v2: split PE/DVE reduction paths.

Key ideas vs baseline (131.4us):
  - Sigma-Delta (error-feedback) quantization on host makes 1-byte dtypes
    essentially exact for segment SUMS (per-seg error <= 1 quant step), so
    x ships as fp8-e3m4 (PE path) / int8 (DVE path): DMA halves to ~18MB/core.
  - Work splits across both big engines:
      PE path  : onehot matmuls on raw nodes (measured ~52ns per 128-node
                 LDW+MM pair), onehots built on DVE at 2x via a transposed
                 c-major is_equal layout.
      DVE path : 3-pass pairwise tree reduce (8 nodes -> 1 group) in
                 int8->int16->int16->fp16 (passes 2-3 run at 2x), then
                 fp16 onehot matmuls on 8x-fewer rows.
  - Per-seg 1/count (and the int8 scale h) folded into the ACT epilogue
    scale vector; bias via ones-column + [weight.T; bias] fp16 GEMM.
Layout: 2048 segs/core = 16 groups x 128 segs; each group = 2 halves of 64
segs; each half is either PE (2 blocks of 32 segs) or DVE (one superblock).
"""
import numpy as np

N_NODES = 2_000_000
IN_CH = 64
OUT_CH = 128
NUM_GRAPHS = 16384
N_CORES = 8
P = 128
SEGS_PER_CORE = NUM_GRAPHS // N_CORES   # 2048
NGROUP = SEGS_PER_CORE // P             # 16 groups of 128 segs
NHALF = 2 * NGROUP                      # 32 halves of 64 segs per core

# halves marked True go to the PE path (2 blocks of 32 segs each);
# False halves go to the DVE path (one 64-seg superblock).
H_PE = 15                               # PE halves per core (tunable 0..32)
LOOKAHEAD = 3                           # groups of produce-ahead
FRONT_PE = 4                            # PE halves pinned to the front
OH2_ON_GPSIMD = False                   # build stage2 onehots on gpsimd
OH1_ON_GPSIMD = False                   # build PE-path onehots on gpsimd
P23_ON_GPSIMD = False                   # run DVE passes 2-3 on gpsimd

TRACE = False
LAST_RESULT = None
_BUILD_CACHE = {}


def _pattern():
    # front-load FRONT_PE PE-halves (PE work available immediately at start),
    # spread the rest evenly over the remaining slots
    front = min(FRONT_PE, H_PE)
    rest = H_PE - front
    nrest = NHALF - front
    pat = [True] * front
    acc = 0
    for i in range(nrest):
        acc += rest
        if acc >= nrest:
            acc -= nrest
            pat.append(True)
        else:
            pat.append(False)
    assert sum(pat) == H_PE and len(pat) == NHALF
    return pat


def _build(T_PE, T2, pat):
    from contextlib import ExitStack
    import concourse.bass as bass
    import concourse.bacc as bacc
    import concourse.tile as tile
    from concourse import mybir

    n_peb = 2 * sum(pat)            # PE blocks per core
    n_sb = NHALF - sum(pat)         # DVE superblocks per core

    nc = bacc.Bacc("TRN2", target_bir_lowering=False, debug=False,
                   num_devices=N_CORES)
    xpe = nc.dram_tensor("xpe", [P, max(1, n_peb) * T_PE * IN_CH],
                         mybir.dt.float8e3, kind="ExternalInput").ap()
    srelpe = nc.dram_tensor("srelpe", [P, max(1, n_peb) * T_PE],
                            mybir.dt.bfloat16, kind="ExternalInput").ap()
    xdv = nc.dram_tensor("xdv", [P, max(1, n_sb) * T2 * 512],
                         mybir.dt.int8, kind="ExternalInput").ap()
    sreldv = nc.dram_tensor("sreldv", [P, max(1, n_sb) * T2],
                            mybir.dt.bfloat16, kind="ExternalInput").ap()
    iota1 = nc.dram_tensor("iota1", [P, 32 * T_PE], mybir.dt.bfloat16,
                           kind="ExternalInput").ap()
    iota2 = nc.dram_tensor("iota2", [P, 64 * T2], mybir.dt.bfloat16,
                           kind="ExternalInput").ap()
    wb = nc.dram_tensor("wb", [IN_CH + 1, OUT_CH], mybir.dt.float16,
                        kind="ExternalInput").ap()
    ident = nc.dram_tensor("ident", [P, P], mybir.dt.float16,
                           kind="ExternalInput").ap()
    scale = nc.dram_tensor("scale", [P, NGROUP], mybir.dt.float32,
                           kind="ExternalInput").ap()
    out = nc.dram_tensor("out", [SEGS_PER_CORE, OUT_CH], mybir.dt.float32,
                         kind="ExternalOutput").ap()

    def ap2(t_, off, d1):
        return bass.AP(tensor=t_.tensor, offset=t_.offset + off,
                       ap=[t_.ap[0], d1])

    def ap3(t_, off, d1, d2):
        return bass.AP(tensor=t_.tensor, offset=t_.offset + off,
                       ap=[t_.ap[0], d1, d2])

    with tile.TileContext(nc) as tc, ExitStack() as ctx:
        singles = ctx.enter_context(tc.tile_pool(name="singles", bufs=1))
        pe_pool = ctx.enter_context(tc.tile_pool(name="pe_pool", bufs=10))
        oh1pool = ctx.enter_context(tc.tile_pool(name="oh1pool", bufs=10))
        dvpool = ctx.enter_context(tc.tile_pool(name="dvpool", bufs=6))
        m1pool = ctx.enter_context(tc.tile_pool(name="m1pool", bufs=3))
        m2pool = ctx.enter_context(tc.tile_pool(name="m2pool", bufs=3))
        rdpool = ctx.enter_context(tc.tile_pool(name="rdpool", bufs=6))
        oh2pool = ctx.enter_context(tc.tile_pool(name="oh2pool", bufs=6))
        meanpool = ctx.enter_context(tc.tile_pool(name="meanpool", bufs=2))
        sbtpool = ctx.enter_context(tc.tile_pool(name="sbtpool", bufs=2))
        outpool = ctx.enter_context(tc.tile_pool(name="outpool", bufs=2))
        psum_acc = ctx.enter_context(
            tc.tile_pool(name="psum_acc", bufs=3, space="PSUM"))
        psum_t = ctx.enter_context(
            tc.tile_pool(name="psum_t", bufs=2, space="PSUM"))
        psum_o = ctx.enter_context(
            tc.tile_pool(name="psum_o", bufs=2, space="PSUM"))

        srelpe_sb = singles.tile([P, max(1, n_peb) * T_PE], mybir.dt.bfloat16)
        nc.sync.dma_start(srelpe_sb, srelpe)
        sreldv_sb = singles.tile([P, max(1, n_sb) * T2], mybir.dt.bfloat16)
        nc.sync.dma_start(sreldv_sb, sreldv)
        iota1_sb = singles.tile([P, 32 * T_PE], mybir.dt.bfloat16)
        nc.sync.dma_start(iota1_sb, iota1)
        iota2_sb = singles.tile([P, 64 * T2], mybir.dt.bfloat16)
        nc.sync.dma_start(iota2_sb, iota2)
        wb_sb = singles.tile([IN_CH + 1, OUT_CH], mybir.dt.float16)
        nc.scalar.dma_start(wb_sb, wb)
        ident_sb = singles.tile([P, P], mybir.dt.float16)
        nc.scalar.dma_start(ident_sb, ident)
        scale_sb = singles.tile([P, NGROUP], mybir.dt.float32)
        nc.scalar.dma_start(scale_sb, scale)

        accs = {}

        def epilogue(g):
            acc = accs.pop(g)
            means = meanpool.tile([P, IN_CH + 1], mybir.dt.float16)
            nc.scalar.activation(
                means[:, 0:IN_CH], acc,
                mybir.ActivationFunctionType.Copy, bias=0.0,
                scale=scale_sb[:, g:g + 1])
            nc.gpsimd.memset(means[:, IN_CH:IN_CH + 1], 1.0)
            pt = psum_t.tile([IN_CH + 1, P], mybir.dt.float16)
            nc.tensor.transpose(pt, means, ident_sb)
            sbt = sbtpool.tile([IN_CH + 1, P], mybir.dt.float16)
            nc.scalar.copy(sbt, pt)
            po = psum_o.tile([P, OUT_CH], mybir.dt.float32)
            nc.tensor.matmul(po, lhsT=sbt, rhs=wb_sb, start=True, stop=True)
            osb = outpool.tile([P, OUT_CH], mybir.dt.float32)
            nc.scalar.copy(osb, po)
            nc.gpsimd.dma_start(out[g * P:(g + 1) * P, :], osb)

        pe_ids = {}   # hidx -> (k0, k1) PE block slab indices
        dv_ids = {}   # hidx -> k superblock slab index
        pe_i = 0
        dv_i = 0
        for hidx in range(NHALF):
            if pat[hidx]:
                pe_ids[hidx] = (pe_i, pe_i + 1)
                pe_i += 2
            else:
                dv_ids[hidx] = dv_i
                dv_i += 1

        produced = {}  # hidx -> dict of tiles for consume
        dma_i = 0

        def produce(hidx):
            nonlocal dma_i
            if hidx >= NHALF:
                return
            ring = nc.sync if dma_i % 2 == 0 else nc.scalar
            dma_i += 1
            if pat[hidx]:
                tiles = []
                for k in pe_ids[hidx]:
                    xs = pe_pool.tile([P, T_PE * IN_CH], mybir.dt.float8e3,
                                      name="xs")
                    ring.dma_start(
                        xs, xpe[:, k * T_PE * IN_CH:(k + 1) * T_PE * IN_CH])
                    oh = oh1pool.tile([P, 32 * T_PE], mybir.dt.bfloat16,
                                      name="oh")
                    oh1_eng = nc.gpsimd if OH1_ON_GPSIMD else nc.vector
                    oh1_eng.tensor_tensor(
                        oh, iota1_sb,
                        ap3(srelpe_sb, k * T_PE, [0, 32], [1, T_PE]),
                        mybir.AluOpType.is_equal)
                    tiles.append((xs, oh))
                produced[hidx] = tiles
            else:
                k = dv_ids[hidx]
                xd = dvpool.tile([P, T2 * 512], mybir.dt.int8, name="xd")
                ring.dma_start(xd, xdv[:, k * T2 * 512:(k + 1) * T2 * 512])
                oh2 = oh2pool.tile([P, 64 * T2], mybir.dt.float16, name="oh2")
                oh2_eng = nc.gpsimd if OH2_ON_GPSIMD else nc.vector
                oh2_eng.tensor_tensor(
                    oh2, iota2_sb,
                    ap3(sreldv_sb, k * T2, [0, 64], [1, T2]),
                    mybir.AluOpType.is_equal)
                npairs1 = T2 * 512 // 128
                m1 = m1pool.tile([P, T2 * 256], mybir.dt.int16, name="m1")
                hq = npairs1 // 2
                nc.vector.tensor_tensor(
                    m1[:, 0:hq * 64], ap3(xd, 0, [128, hq], [1, 64]),
                    ap3(xd, 64, [128, hq], [1, 64]), mybir.AluOpType.add)
                nc.vector.tensor_tensor(
                    m1[:, hq * 64:], ap3(xd, hq * 128, [128, hq], [1, 64]),
                    ap3(xd, hq * 128 + 64, [128, hq], [1, 64]),
                    mybir.AluOpType.add)
                p23_eng = nc.gpsimd if P23_ON_GPSIMD else nc.vector
                m2 = m2pool.tile([P, T2 * 128], mybir.dt.int16, name="m2")
                p23_eng.tensor_tensor(
                    m2, ap3(m1, 0, [128, npairs1 // 2], [1, 64]),
                    ap3(m1, 64, [128, npairs1 // 2], [1, 64]),
                    mybir.AluOpType.add)
                rd = rdpool.tile([P, T2 * IN_CH], mybir.dt.float16, name="rd")
                p23_eng.tensor_tensor(
                    rd, ap3(m2, 0, [128, npairs1 // 4], [1, 64]),
                    ap3(m2, 64, [128, npairs1 // 4], [1, 64]),
                    mybir.AluOpType.add)
                produced[hidx] = (rd, oh2)

        def consume(hidx, acc_g):
            g, half = hidx // 2, hidx % 2
            if pat[hidx]:
                for bi, (xs, oh) in enumerate(produced.pop(hidx)):
                    row = 64 * half + 32 * bi
                    dst = acc_g[row:row + 32, :]
                    for t in range(T_PE):
                        nc.tensor.matmul(
                            dst,
                            lhsT=ap2(oh, t, [T_PE, 32]),
                            rhs=xs[:, t * IN_CH:(t + 1) * IN_CH],
                            start=(t == 0), stop=(t == T_PE - 1),
                            tile_position=(0, row))
            else:
                rd, oh2 = produced.pop(hidx)
                row = 64 * half
                dst = acc_g[row:row + 64, :]
                for t2 in range(T2):
                    nc.tensor.matmul(
                        dst,
                        lhsT=ap2(oh2, t2, [T2, 64]),
                        rhs=rd[:, t2 * IN_CH:(t2 + 1) * IN_CH],
                        start=(t2 == 0), stop=(t2 == T2 - 1),
                        tile_position=(0, row))

        for h in range(2 * LOOKAHEAD):
            produce(h)
        for g in range(NGROUP):
            acc_g = psum_acc.tile([P, IN_CH], mybir.dt.float32, name="acc_g")
            accs[g] = acc_g
            for half in range(2):
                hidx = 2 * g + half
                consume(hidx, acc_g)
                produce(hidx + 2 * LOOKAHEAD)
            if g >= 1:
                epilogue(g - 1)
        epilogue(NGROUP - 1)
    nc.compile()
    return nc


def _sigma_delta_fp8(xpad, valid):
    """Native-grid e3m4 error-feedback quantization along axis 1."""
    import ml_dtypes
    S, L, F = xpad.shape
    q = np.zeros((S, L, F), ml_dtypes.float8_e3m4)
    delta = np.zeros((S, F), np.float32)
    for j in range(L):
        m = valid[:, j][:, None]
        a = xpad[:, j, :] + delta
        qj = a.astype(ml_dtypes.float8_e3m4)
        qf = qj.astype(np.float32)
        q[:, j, :] = np.where(m, qj, np.zeros((), ml_dtypes.float8_e3m4))
        delta = np.where(m, a - qf, delta)
    return q


def _sigma_delta_int8(xpad, h):
    """Uniform-grid error-feedback quantization via per-seg cumsum."""
    c = np.cumsum(xpad.astype(np.float64) / h, axis=1)
    r = np.round(c)
    q = np.diff(r, axis=1, prepend=0.0)
    assert np.abs(q).max() <= 127
    return q.astype(np.int8)


def _ensure_ntff_hook():
    import sys
    import types
    try:
        import antenv.axon_hooks  # noqa: F401
        return
    except ImportError:
        pass
    import antenv
    mod = types.ModuleType("antenv.axon_hooks")
    holder = {"h": None}
    mod.set_axon_ntff_profile_hook = lambda h: holder.__setitem__("h", h)
    mod.get_axon_ntff_profile_hook = lambda: holder["h"]
    sys.modules["antenv.axon_hooks"] = mod
    antenv.axon_hooks = mod
    try:
        from trn_agent_boot.trn_boot import _ntff_profile_via_ctypes
        mod.set_axon_ntff_profile_hook(
            _ntff_profile_via_ctypes("/opt/axon/libaxon_pjrt.so"))
    except Exception as e:
        print(f"ntff hook unavailable: {e}")


def kernel(x, segment_ids, weight, bias, num_graphs):
    global LAST_RESULT
    import ml_dtypes
    from concourse import bass_utils
    if TRACE:
        _ensure_ntff_hook()

    f8 = ml_dtypes.float8_e3m4
    bf16 = ml_dtypes.bfloat16
    x = np.asarray(x, dtype=np.float32)
    seg = np.asarray(segment_ids).astype(np.int64)
    weight = np.asarray(weight, dtype=np.float32)
    bias = np.asarray(bias, dtype=np.float32)
    G = int(num_graphs)
    assert G == NUM_GRAPHS and x.shape == (N_NODES, IN_CH)

    pat = _pattern()
    bounds = np.searchsorted(seg, np.arange(G + 1))
    cnts = np.diff(bounds).astype(np.int64)
    h = float(np.abs(x).max()) / 126.0

    # padded per-seg array [G, L, F] (zeros at pad)
    L = int(((cnts + 7) // 8).max() * 8)
    idx_in_seg = np.arange(N_NODES) - bounds[seg]
    # seg path assignment: seg s -> half (s % 2048) // 64 of its core
    half_of_seg = (np.arange(G) % SEGS_PER_CORE) // 64
    seg_is_pe = np.array(pat, bool)[half_of_seg]

    # ---- PE path quantization (e3m4 native grid, sequential feedback)
    pe_segs = np.where(seg_is_pe)[0]
    dv_segs = np.where(~seg_is_pe)[0]
    pe_rank = np.full(G, -1, np.int64)
    pe_rank[pe_segs] = np.arange(len(pe_segs))
    dv_rank = np.full(G, -1, np.int64)
    dv_rank[dv_segs] = np.arange(len(dv_segs))

    node_pe = seg_is_pe[seg]
    q_pe = None
    if len(pe_segs):
        xpad = np.zeros((len(pe_segs), L, IN_CH), np.float32)
        vpad = np.zeros((len(pe_segs), L), bool)
        xpad[pe_rank[seg[node_pe]], idx_in_seg[node_pe]] = x[node_pe]
        vpad[pe_rank[seg[node_pe]], idx_in_seg[node_pe]] = True
        q_pe = _sigma_delta_fp8(xpad, vpad)  # [n_pe_segs, L, F] e3m4
        del xpad, vpad
    q_dv = None
    if len(dv_segs):
        xpad = np.zeros((len(dv_segs), L, IN_CH), np.float32)
        xpad[dv_rank[seg[~node_pe]], idx_in_seg[~node_pe]] = x[~node_pe]
        q_dv = _sigma_delta_int8(xpad, h)    # [n_dv_segs, L, F] int8
        del xpad

    # ---- geometry
    n_pe_half = sum(pat)
    n_peb = 2 * n_pe_half
    n_sb = NHALF - n_pe_half
    cnts_core = cnts.reshape(N_CORES, SEGS_PER_CORE)

    # per (core, block32): node count; block b covers segs [32b, 32b+32)
    blk_cnt = cnts_core.reshape(N_CORES, 64, 32).sum(axis=2)
    pat_arr = np.array(pat, bool)
    blk_is_pe = pat_arr[(np.arange(64)) // 2]
    T_PE = int(np.ceil(blk_cnt[:, blk_is_pe].max() / P)) if n_peb else 1
    # per (core, sblock64): group-slot count = sum ceil(cnt/8) over 64 segs
    g8 = (cnts_core + 7) // 8
    sb_groups = g8.reshape(N_CORES, 32, 64).sum(axis=2)  # per half window
    sb_is_dv = ~pat_arr
    T2 = int(np.ceil(sb_groups[:, sb_is_dv].max() / P)) if n_sb else 1

    # ---- assemble per-core slabs
    xpe_all = np.zeros((N_CORES, P, max(1, n_peb) * T_PE * IN_CH), f8)
    srelpe_all = np.full((N_CORES, P, max(1, n_peb) * T_PE), -1.0, np.float32)
    xdv_all = np.zeros((N_CORES, P, max(1, n_sb) * T2 * 512), np.int8)
    sreldv_all = np.full((N_CORES, P, max(1, n_sb) * T2), -1.0, np.float32)

    for c in range(N_CORES):
        pe_k = 0
        dv_k = 0
        for hidx in range(NHALF):
            segs0 = c * SEGS_PER_CORE + hidx * 64  # first seg of half
            if pat[hidx]:
                for bi in range(2):
                    sa = segs0 + 32 * bi
                    nb = int(blk_cnt[c, hidx * 2 + bi])
                    # gather quantized nodes of segs [sa, sa+32) in order
                    rows = np.zeros((T_PE * P, IN_CH), f8)
                    srl = np.full(T_PE * P, -1.0, np.float32)
                    o = 0
                    for s in range(sa, sa + 32):
                        n = int(cnts[s])
                        rows[o:o + n] = q_pe[pe_rank[s], :n]
                        srl[o:o + n] = s - sa
                        o += n
                    assert o == nb <= T_PE * P
                    xpe_all[c, :, pe_k * T_PE * IN_CH:(pe_k + 1) * T_PE * IN_CH] = (
                        rows.reshape(T_PE, P, IN_CH).transpose(1, 0, 2)
                        .reshape(P, T_PE * IN_CH))
                    srelpe_all[c, :, pe_k * T_PE:(pe_k + 1) * T_PE] = (
                        srl.reshape(T_PE, P).T)
                    pe_k += 1
            else:
                slots = np.zeros((T2 * P, 8, IN_CH), np.int8)
                srl = np.full(T2 * P, -1.0, np.float32)
                o = 0
                for s in range(segs0, segs0 + 64):
                    ng = int(g8[c, s - c * SEGS_PER_CORE])
                    qq = q_dv[dv_rank[s], :ng * 8].reshape(ng, 8, IN_CH)
                    slots[o:o + ng] = qq
                    srl[o:o + ng] = s - segs0
                    o += ng
                assert o <= T2 * P
                xdv_all[c, :, dv_k * T2 * 512:(dv_k + 1) * T2 * 512] = (
                    slots.reshape(T2, P, 8 * IN_CH).transpose(1, 0, 2)
                    .reshape(P, T2 * 512))
                sreldv_all[c, :, dv_k * T2:(dv_k + 1) * T2] = (
                    srl.reshape(T2, P).T)
                dv_k += 1

    # iota tables (c-major to keep is_equal inner steps = 1)
    i1 = np.zeros((P, 32 * T_PE), np.float32)
    i1[:, :] = np.repeat(np.arange(32, dtype=np.float32), T_PE)[None, :]
    i2 = np.zeros((P, 64 * T2), np.float32)
    i2[:, :] = np.repeat(np.arange(64, dtype=np.float32), T2)[None, :]

    wb = np.concatenate([weight.T, bias[None]], axis=0).astype(np.float16)
    ident = np.eye(P, dtype=np.float16)
    # epilogue scale: 1/cnt, with int8 step h folded in for DVE segs
    sc = 1.0 / cnts_core.astype(np.float32)
    sc = np.where(seg_is_pe.reshape(N_CORES, SEGS_PER_CORE), sc, sc * h)
    # [core, 2048] -> [core, P, NGROUP]: seg 128g + p
    scale_all = np.ascontiguousarray(
        sc.reshape(N_CORES, NGROUP, P).transpose(0, 2, 1)).astype(np.float32)

    key = (T_PE, T2, tuple(pat))
    if key not in _BUILD_CACHE:
        _BUILD_CACHE[key] = _build(T_PE, T2, pat)
    nc = _BUILD_CACHE[key]

    in_maps = [
        dict(xpe=xpe_all[c], srelpe=srelpe_all[c].astype(bf16),
             xdv=xdv_all[c], sreldv=sreldv_all[c].astype(bf16),
             iota1=i1.astype(bf16), iota2=i2.astype(bf16),
             wb=wb, ident=ident, scale=scale_all[c])
        for c in range(N_CORES)
    ]
    res = bass_utils.run_bass_kernel_spmd(
        nc, in_maps, core_ids=list(range(N_CORES)), trace=TRACE)
    LAST_RESULT = res
    return np.concatenate(
        [res.results[c]["out"] for c in range(N_CORES)], axis=0
    ).astype(np.float32)



# revision 5
# speedup vs baseline: 1.5257x; 1.5257x over previous
"""DrugGNN segment-mean + linear embed, v4: all-PE DoubleRow design.

Architecture (per core, 2048 segs = 16 groups x 128 segs = 64 blocks x 32):
  - Host pads every segment count to a multiple of 8 ("slot rows" of 8
    nodes), snake-packs segments into 512 (core, block) bins of exactly 32
    segments each so every block has <= T*128 = 512 slot rows, and
    sigma-delta quantizes x on the fp8-e4m3 grid (error feedback makes
    per-segment sums exact to ~1 quant step).
  - Block slab layout [128p, T*512]: Q-group g occupies cols [g*512,
    (g+1)*512); its 8 tiles of 64 channels share ONE onehot pattern
    (row -> local seg), so each Q-group is a single DoubleRow matmul:
    lhsT = onehot [128, (0,2),(1,32)] fp8e4 (stride-0 k-tile share),
    rhs = slab [128, (64,2),(128,4),(1,64)], out = acc[strip:strip+32]
    with stride-0 free AP [(0,4),(1,64)] accumulating all 4 pairs into
    the same PSUM columns. 131ns per 1024 nodes measured.
  - Onehots built on DVE: is_equal(iota[128,T*32], srel bcast [(1,T),(0,32)]).
  - Epilogue per group: ACT scale (1/cnt) -> fp16 means + ones col, PE
    transpose, fp16 GEMM with [weight.T; bias], DMA out. Host un-permutes
    rows at the end.
"""
import numpy as np

N_NODES = 2_000_000
IN_CH = 64
OUT_CH = 128
NUM_GRAPHS = 16384
N_CORES = 8
P = 128
SEGS_PER_CORE = NUM_GRAPHS // N_CORES   # 2048
NGROUP = SEGS_PER_CORE // P             # 16 groups of 128 segs
NBLK = 4 * NGROUP                       # 64 blocks of 32 segs per core
NBIN = N_CORES * NBLK                   # 512 bins globally
LOOKAHEAD = 8                           # blocks of produce-ahead

TRACE = False
LAST_RESULT = None
_BUILD_CACHE = {}


def _build(T):
    from contextlib import ExitStack
    import concourse.bass as bass
    import concourse.bacc as bacc
    import concourse.tile as tile
    from concourse import mybir

    nc = bacc.Bacc("TRN2", target_bir_lowering=False, debug=False,
                   num_devices=N_CORES)
    dt = mybir.dt
    xq = nc.dram_tensor("xq", [P, NBLK * T * 512], dt.float8e3,
                        kind="ExternalInput").ap()
    srel = nc.dram_tensor("srel", [P, NBLK * T], dt.bfloat16,
                          kind="ExternalInput").ap()
    iota = nc.dram_tensor("iota", [P, T * 32], dt.bfloat16,
                          kind="ExternalInput").ap()
    wb = nc.dram_tensor("wb", [IN_CH + 1, OUT_CH], dt.float16,
                        kind="ExternalInput").ap()
    ident = nc.dram_tensor("ident", [P, P], dt.float16,
                           kind="ExternalInput").ap()
    scale = nc.dram_tensor("scale", [P, NGROUP], dt.float32,
                           kind="ExternalInput").ap()
    out = nc.dram_tensor("out", [SEGS_PER_CORE, OUT_CH], dt.float32,
                         kind="ExternalOutput").ap()

    def ap3(t_, off, d1, d2):
        return bass.AP(tensor=t_.tensor, offset=t_.offset + off,
                       ap=[t_.ap[0], d1, d2])

    def ap4(t_, off, d1, d2, d3):
        return bass.AP(tensor=t_.tensor, offset=t_.offset + off,
                       ap=[t_.ap[0], d1, d2, d3])

    with tile.TileContext(nc) as tc, ExitStack() as ctx:
        singles = ctx.enter_context(tc.tile_pool(name="singles", bufs=1))
        slabs = ctx.enter_context(
            tc.tile_pool(name="slabs", bufs=LOOKAHEAD + 3))
        ohpool = ctx.enter_context(
            tc.tile_pool(name="ohpool", bufs=LOOKAHEAD + 3))
        meanpool = ctx.enter_context(tc.tile_pool(name="meanpool", bufs=2))
        sbtpool = ctx.enter_context(tc.tile_pool(name="sbtpool", bufs=2))
        outpool = ctx.enter_context(tc.tile_pool(name="outpool", bufs=2))
        psum_acc = ctx.enter_context(
            tc.tile_pool(name="psum_acc", bufs=3, space="PSUM"))
        psum_t = ctx.enter_context(
            tc.tile_pool(name="psum_t", bufs=2, space="PSUM"))
        psum_o = ctx.enter_context(
            tc.tile_pool(name="psum_o", bufs=2, space="PSUM"))

        srel_sb = singles.tile([P, NBLK * T], dt.bfloat16, name="srel")
        nc.sync.dma_start(srel_sb, srel)
        iota_sb = singles.tile([P, T * 32], dt.bfloat16, name="iota")
        nc.sync.dma_start(iota_sb, iota)
        wb_sb = singles.tile([IN_CH + 1, OUT_CH], dt.float16, name="wb")
        nc.scalar.dma_start(wb_sb, wb)
        ident_sb = singles.tile([P, P], dt.float16, name="ident")
        nc.scalar.dma_start(ident_sb, ident)
        scale_sb = singles.tile([P, NGROUP], dt.float32, name="scale")
        nc.scalar.dma_start(scale_sb, scale)

        accs = {}

        def epilogue(g):
            acc = accs.pop(g)
            means = meanpool.tile([P, IN_CH + 1], dt.float16)
            nc.scalar.activation(
                means[:, 0:IN_CH], acc,
                mybir.ActivationFunctionType.Copy, bias=0.0,
                scale=scale_sb[:, g:g + 1])
            nc.gpsimd.memset(means[:, IN_CH:IN_CH + 1], 1.0)
            pt = psum_t.tile([IN_CH + 1, P], dt.float16)
            nc.tensor.transpose(pt, means, ident_sb)
            sbt = sbtpool.tile([IN_CH + 1, P], dt.float16)
            nc.scalar.copy(sbt, pt)
            po = psum_o.tile([P, OUT_CH], dt.float32)
            nc.tensor.matmul(po, lhsT=sbt, rhs=wb_sb, start=True, stop=True)
            osb = outpool.tile([P, OUT_CH], dt.float32)
            nc.scalar.copy(osb, po)
            nc.gpsimd.dma_start(out[g * P:(g + 1) * P, :], osb)

        produced = {}
        dma_i = 0

        def produce(b):
            nonlocal dma_i
            if b >= NBLK:
                return
            ring = (nc.sync, nc.scalar, nc.gpsimd)[dma_i % 3]
            dma_i += 1
            xs = slabs.tile([P, T * 512], dt.float8e3, name="xs")
            ring.dma_start(xs, xq[:, b * T * 512:(b + 1) * T * 512])
            oh = ohpool.tile([P, T * 32], dt.float8e3, name="oh")
            nc.vector.tensor_tensor(
                oh, iota_sb, ap3(srel_sb, b * T, [1, T], [0, 32]),
                mybir.AluOpType.is_equal)
            produced[b] = (xs, oh)

        def consume(b):
            g_idx = b // 4
            strip = 32 * (b % 4)
            xs, oh = produced.pop(b)
            acc = accs[g_idx]
            sl = acc[strip:strip + 32, :]
            dst = bass.AP(tensor=sl.tensor, offset=sl.offset,
                          ap=[sl.ap[0], [0, 8], [1, IN_CH]])
            for g in range(T):
                nc.tensor.matmul(
                    dst,
                    lhsT=oh[:, g * 32:(g + 1) * 32],
                    rhs=xs[:, g * 512:(g + 1) * 512],
                    start=(g == 0), stop=(g == T - 1),
                    tile_position=(0, strip))

        for b in range(LOOKAHEAD):
            produce(b)
        for g_idx in range(NGROUP):
            accs[g_idx] = psum_acc.tile([P, IN_CH], dt.float32, name="acc")
            for j in range(4):
                b = 4 * g_idx + j
                consume(b)
                produce(b + LOOKAHEAD)
            if g_idx >= 1:
                epilogue(g_idx - 1)
        epilogue(NGROUP - 1)
    nc.compile()
    return nc


def _sigma_delta_fp8(xpad, valid, qdtype):
    """Native-grid error-feedback quantization along axis 1."""
    S, L, F = xpad.shape
    q = np.zeros((S, L, F), qdtype)
    delta = np.zeros((S, F), np.float32)
    for j in range(L):
        m = valid[:, j][:, None]
        a = xpad[:, j, :] + delta
        qj = a.astype(qdtype)
        qf = qj.astype(np.float32)
        q[:, j, :] = np.where(m, qj, np.zeros((), qdtype))
        delta = np.where(m, a - qf, delta)
    return q


def _ensure_ntff_hook():
    import sys
    import types
    try:
        import antenv.axon_hooks  # noqa: F401
        return
    except ImportError:
        pass
    import antenv
    mod = types.ModuleType("antenv.axon_hooks")
    holder = {"h": None}
    mod.set_axon_ntff_profile_hook = lambda h: holder.__setitem__("h", h)
    mod.get_axon_ntff_profile_hook = lambda: holder["h"]
    sys.modules["antenv.axon_hooks"] = mod
    antenv.axon_hooks = mod
    try:
        from trn_agent_boot.trn_boot import _ntff_profile_via_ctypes
        mod.set_axon_ntff_profile_hook(
            _ntff_profile_via_ctypes("/opt/axon/libaxon_pjrt.so"))
    except Exception as e:
        print(f"ntff hook unavailable: {e}")


def kernel(x, segment_ids, weight, bias, num_graphs):
    global LAST_RESULT
    import ml_dtypes
    from concourse import bass_utils

    if TRACE:
        _ensure_ntff_hook()

    f8e4 = ml_dtypes.float8_e3m4
    bf16 = ml_dtypes.bfloat16
    x = np.asarray(x, dtype=np.float32)
    seg = np.asarray(segment_ids).astype(np.int64)
    weight = np.asarray(weight, dtype=np.float32)
    bias = np.asarray(bias, dtype=np.float32)
    G = int(num_graphs)
    assert G == NUM_GRAPHS and x.shape == (N_NODES, IN_CH)

    bounds = np.searchsorted(seg, np.arange(G + 1))
    cnts = np.diff(bounds).astype(np.int64)
    m = (cnts + 7) // 8                      # slot rows per seg

    # ---- snake-pack segments into 512 bins of exactly 32 segs ----
    order = np.argsort(-m, kind="stable")
    bin_of_seg = np.empty(G, np.int64)
    local_of_seg = np.empty(G, np.int64)
    fwd = np.arange(NBIN)
    rev = fwd[::-1]
    for r in range(G // NBIN):               # 32 rounds
        rowsegs = order[r * NBIN:(r + 1) * NBIN]
        bins = fwd if r % 2 == 0 else rev
        bin_of_seg[rowsegs] = bins
        local_of_seg[rowsegs] = r
    R = np.zeros(NBIN, np.int64)
    np.add.at(R, bin_of_seg, m)
    T = int(np.ceil(R.max() / P))
    assert T * P >= R.max()

    # per-seg starting slot row within its block (assignment order per bin)
    row_start = np.zeros(G, np.int64)
    base = np.zeros(NBIN, np.int64)
    for r in range(G // NBIN):
        rowsegs = order[r * NBIN:(r + 1) * NBIN]
        b = bin_of_seg[rowsegs]
        row_start[rowsegs] = base[b]
        base[b] += m[rowsegs]

    # ---- sigma-delta quantize on e4m3 grid ----
    L = int(m.max() * 8)
    idx_in_seg = np.arange(N_NODES) - bounds[seg]
    xpad = np.zeros((G, L, IN_CH), np.float32)
    vpad = np.zeros((G, L), bool)
    xpad[seg, idx_in_seg] = x
    vpad[seg, idx_in_seg] = True
    q = _sigma_delta_fp8(xpad, vpad, f8e4)   # [G, L, F]
    del xpad, vpad
    q = q.reshape(G, L // 8, 8, IN_CH)

    # ---- scatter into per-core slabs ----
    # per slot row: seg, row index within block
    seg_rep = np.repeat(np.arange(G), m)                    # [Rtot]
    csum = np.concatenate([[0], np.cumsum(m)])
    r_in_seg = np.arange(len(seg_rep)) - csum[seg_rep]      # [Rtot]
    row_blk = row_start[seg_rep] + r_in_seg                 # block row
    bin_r = bin_of_seg[seg_rep]
    core_r = bin_r // NBLK
    blk_r = bin_r % NBLK
    g_r = row_blk // P
    p_r = row_blk % P

    xq_all = np.zeros((N_CORES, P, NBLK * T * 512), f8e4)
    vals = q[seg_rep, r_in_seg]                             # [Rtot, 8, F]
    cols = (blk_r * (T * 512) + g_r * 512)[:, None, None] + \
        (np.arange(8) * IN_CH)[None, :, None] + \
        np.arange(IN_CH)[None, None, :]
    xq_all[core_r[:, None, None], p_r[:, None, None], cols] = vals
    del q, vals, cols

    srel_all = np.full((N_CORES, P, NBLK * T), -1.0, np.float32)
    srel_all[core_r, p_r, blk_r * T + g_r] = local_of_seg[seg_rep]

    iota = np.tile(np.arange(32, dtype=np.float32), T)[None].repeat(P, 0)

    # ---- epilogue scale + output permutation ----
    # device row (core, grp*128 + p) holds seg with bin=core*NBLK+grp*4+p//32,
    # local=p%32
    grp = np.arange(SEGS_PER_CORE) // P
    p_of = np.arange(SEGS_PER_CORE) % P
    seg_at = np.empty((N_CORES, SEGS_PER_CORE), np.int64)
    inv = np.empty(G, np.int64)
    inv[bin_of_seg * 32 + local_of_seg] = np.arange(G)
    for c in range(N_CORES):
        bins_ = c * NBLK + grp * 4 + p_of // 32
        seg_at[c] = inv[bins_ * 32 + p_of % 32]
    sc = 1.0 / np.maximum(cnts, 1).astype(np.float32)[seg_at]  # [C, 2048]
    scale_all = np.ascontiguousarray(
        sc.reshape(N_CORES, NGROUP, P).transpose(0, 2, 1)).astype(np.float32)

    wb = np.concatenate([weight.T, bias[None]], axis=0).astype(np.float16)
    ident = np.eye(P, dtype=np.float16)

    if T not in _BUILD_CACHE:
        _BUILD_CACHE[T] = _build(T)
    nc = _BUILD_CACHE[T]

    in_maps = [
        dict(xq=xq_all[c], srel=srel_all[c].astype(bf16),
             iota=iota.astype(bf16), wb=wb, ident=ident,
             scale=scale_all[c])
        for c in range(N_CORES)
    ]
    res = bass_utils.run_bass_kernel_spmd(
        nc, in_maps, core_ids=list(range(N_CORES)), trace=TRACE)
    LAST_RESULT = res
    dev = np.concatenate(
        [res.results[c]["out"] for c in range(N_CORES)], axis=0)
    out_full = np.empty((G, OUT_CH), np.float32)
    out_full[seg_at.reshape(-1)] = dev.astype(np.float32)
    return out_full


# revision 7
# speedup vs baseline: 1.5461x; 1.0134x over previous
"""DrugGNN segment-mean + linear embed, v4: all-PE DoubleRow design.

Architecture (per core, 2048 segs = 16 groups x 128 segs = 64 blocks x 32):
  - Host pads every segment count to a multiple of 8 ("slot rows" of 8
    nodes), snake-packs segments into 512 (core, block) bins of exactly 32
    segments each so every block has <= T*128 = 512 slot rows, and
    sigma-delta quantizes x on the fp8-e4m3 grid (error feedback makes
    per-segment sums exact to ~1 quant step).
  - Block slab layout [128p, T*512]: Q-group g occupies cols [g*512,
    (g+1)*512); its 8 tiles of 64 channels share ONE onehot pattern
    (row -> local seg), so each Q-group is a single DoubleRow matmul:
    lhsT = onehot [128, (0,2),(1,32)] fp8e4 (stride-0 k-tile share),
    rhs = slab [128, (64,2),(128,4),(1,64)], out = acc[strip:strip+32]
    with stride-0 free AP [(0,4),(1,64)] accumulating all 4 pairs into
    the same PSUM columns. 131ns per 1024 nodes measured.
  - Onehots built on DVE: is_equal(iota[128,T*32], srel bcast [(1,T),(0,32)]).
  - Epilogue per group: ACT scale (1/cnt) -> fp16 means + ones col, PE
    transpose, fp16 GEMM with [weight.T; bias], DMA out. Host un-permutes
    rows at the end.
"""
import numpy as np

N_NODES = 2_000_000
IN_CH = 64
OUT_CH = 128
NUM_GRAPHS = 16384
N_CORES = 8
P = 128
SEGS_PER_CORE = NUM_GRAPHS // N_CORES   # 2048
NGROUP = SEGS_PER_CORE // P             # 16 groups of 128 segs
NBLK = 4 * NGROUP                       # 64 blocks of 32 segs per core
NBIN = N_CORES * NBLK                   # 512 bins globally
LOOKAHEAD = 20                          # blocks of produce-ahead

TRACE = False
LAST_RESULT = None
_BUILD_CACHE = {}


def _build(T):
    from contextlib import ExitStack
    import concourse.bass as bass
    import concourse.bacc as bacc
    import concourse.tile as tile
    from concourse import mybir

    nc = bacc.Bacc("TRN2", target_bir_lowering=False, debug=False,
                   num_devices=N_CORES)
    dt = mybir.dt
    xq = nc.dram_tensor("xq", [P, NBLK * T * 512], dt.float8e3,
                        kind="ExternalInput").ap()
    srel = nc.dram_tensor("srel", [P, NBLK * T], dt.bfloat16,
                          kind="ExternalInput").ap()
    iota = nc.dram_tensor("iota", [P, T * 32], dt.bfloat16,
                          kind="ExternalInput").ap()
    wb = nc.dram_tensor("wb", [IN_CH + 1, OUT_CH], dt.float16,
                        kind="ExternalInput").ap()
    ident = nc.dram_tensor("ident", [P, P], dt.float16,
                           kind="ExternalInput").ap()
    scale = nc.dram_tensor("scale", [P, NGROUP], dt.float32,
                           kind="ExternalInput").ap()
    out = nc.dram_tensor("out", [SEGS_PER_CORE, OUT_CH], dt.float16,
                         kind="ExternalOutput").ap()

    def ap3(t_, off, d1, d2):
        return bass.AP(tensor=t_.tensor, offset=t_.offset + off,
                       ap=[t_.ap[0], d1, d2])

    def ap4(t_, off, d1, d2, d3):
        return bass.AP(tensor=t_.tensor, offset=t_.offset + off,
                       ap=[t_.ap[0], d1, d2, d3])

    with tile.TileContext(nc) as tc, ExitStack() as ctx:
        singles = ctx.enter_context(tc.tile_pool(name="singles", bufs=1))
        slabs = ctx.enter_context(
            tc.tile_pool(name="slabs", bufs=LOOKAHEAD // 2 + 2))
        ohpool = ctx.enter_context(
            tc.tile_pool(name="ohpool", bufs=LOOKAHEAD + 3))
        meanpool = ctx.enter_context(tc.tile_pool(name="meanpool", bufs=2))
        sbtpool = ctx.enter_context(tc.tile_pool(name="sbtpool", bufs=2))
        outpool = ctx.enter_context(tc.tile_pool(name="outpool", bufs=2))
        psum_acc = ctx.enter_context(
            tc.tile_pool(name="psum_acc", bufs=3, space="PSUM"))
        psum_t = ctx.enter_context(
            tc.tile_pool(name="psum_t", bufs=2, space="PSUM"))
        psum_o = ctx.enter_context(
            tc.tile_pool(name="psum_o", bufs=2, space="PSUM"))

        srel_sb = singles.tile([P, NBLK * T], dt.bfloat16, name="srel")
        nc.sync.dma_start(srel_sb, srel)
        iota_sb = singles.tile([P, T * 32], dt.bfloat16, name="iota")
        nc.sync.dma_start(iota_sb, iota)
        wb_sb = singles.tile([IN_CH + 1, OUT_CH], dt.float16, name="wb")
        nc.scalar.dma_start(wb_sb, wb)
        ident_sb = singles.tile([P, P], dt.float16, name="ident")
        nc.scalar.dma_start(ident_sb, ident)
        scale_sb = singles.tile([P, NGROUP], dt.float32, name="scale")
        nc.scalar.dma_start(scale_sb, scale)

        accs = {}

        def epilogue(g):
            acc = accs.pop(g)
            means = meanpool.tile([P, IN_CH + 1], dt.float16)
            nc.scalar.activation(
                means[:, 0:IN_CH], acc,
                mybir.ActivationFunctionType.Copy, bias=0.0,
                scale=scale_sb[:, g:g + 1])
            nc.gpsimd.memset(means[:, IN_CH:IN_CH + 1], 1.0)
            pt = psum_t.tile([IN_CH + 1, P], dt.float16)
            nc.tensor.transpose(pt, means, ident_sb)
            sbt = sbtpool.tile([IN_CH + 1, P], dt.float16)
            nc.scalar.copy(sbt, pt)
            po = psum_o.tile([P, OUT_CH], dt.float32)
            nc.tensor.matmul(po, lhsT=sbt, rhs=wb_sb, start=True, stop=True)
            osb = outpool.tile([P, OUT_CH], dt.float16)
            nc.scalar.copy(osb, po)
            nc.gpsimd.dma_start(out[g * P:(g + 1) * P, :], osb)

        produced = {}
        oh_made = {}
        dma_i = 0

        def produce(sb):
            # one DMA per 2 blocks
            nonlocal dma_i
            if 2 * sb >= NBLK:
                return
            ring = (nc.sync, nc.scalar, nc.gpsimd)[dma_i % 3]
            dma_i += 1
            xs = slabs.tile([P, 2 * T * 512], dt.float8e3, name="xs")
            ring.dma_start(xs, xq[:, 2 * sb * T * 512:(2 * sb + 2) * T * 512])
            produced[sb] = xs

        def make_oh(b):
            if b >= NBLK:
                return
            oh = ohpool.tile([P, T * 32], dt.float8e3, name="oh")
            nc.vector.tensor_tensor(
                oh, iota_sb, ap3(srel_sb, b * T, [1, T], [0, 32]),
                mybir.AluOpType.is_equal)
            oh_made[b] = oh

        def consume(b):
            g_idx = b // 4
            strip = 32 * (b % 4)
            xs = produced[b // 2] if b % 2 == 0 else produced.pop(b // 2)
            off = (b % 2) * T * 512
            oh = oh_made.pop(b)
            acc = accs[g_idx]
            sl = acc[strip:strip + 32, :]
            dst = bass.AP(tensor=sl.tensor, offset=sl.offset,
                          ap=[sl.ap[0], [0, 8], [1, IN_CH]])
            for g in range(T):
                nc.tensor.matmul(
                    dst,
                    lhsT=oh[:, g * 32:(g + 1) * 32],
                    rhs=xs[:, off + g * 512:off + (g + 1) * 512],
                    start=(g == 0), stop=(g == T - 1),
                    tile_position=(0, strip))

        for sb in range(LOOKAHEAD // 2):
            produce(sb)
        for b in range(min(LOOKAHEAD, NBLK)):
            make_oh(b)
        for g_idx in range(NGROUP):
            accs[g_idx] = psum_acc.tile([P, IN_CH], dt.float32, name="acc")
            for j in range(4):
                b = 4 * g_idx + j
                consume(b)
                if (b + LOOKAHEAD) % 2 == 0:
                    produce((b + LOOKAHEAD) // 2)
                make_oh(b + LOOKAHEAD)
            if g_idx >= 1:
                epilogue(g_idx - 1)
        epilogue(NGROUP - 1)
    nc.compile()
    return nc


def _sigma_delta_fp8(xpad, valid, qdtype):
    """Native-grid error-feedback quantization along axis 1."""
    S, L, F = xpad.shape
    q = np.zeros((S, L, F), qdtype)
    delta = np.zeros((S, F), np.float32)
    for j in range(L):
        m = valid[:, j][:, None]
        a = xpad[:, j, :] + delta
        qj = a.astype(qdtype)
        qf = qj.astype(np.float32)
        q[:, j, :] = np.where(m, qj, np.zeros((), qdtype))
        delta = np.where(m, a - qf, delta)
    return q


def _ensure_ntff_hook():
    import sys
    import types
    try:
        import antenv.axon_hooks  # noqa: F401
        return
    except ImportError:
        pass
    import antenv
    mod = types.ModuleType("antenv.axon_hooks")
    holder = {"h": None}
    mod.set_axon_ntff_profile_hook = lambda h: holder.__setitem__("h", h)
    mod.get_axon_ntff_profile_hook = lambda: holder["h"]
    sys.modules["antenv.axon_hooks"] = mod
    antenv.axon_hooks = mod
    try:
        from trn_agent_boot.trn_boot import _ntff_profile_via_ctypes
        mod.set_axon_ntff_profile_hook(
            _ntff_profile_via_ctypes("/opt/axon/libaxon_pjrt.so"))
    except Exception as e:
        print(f"ntff hook unavailable: {e}")


def kernel(x, segment_ids, weight, bias, num_graphs):
    global LAST_RESULT
    import ml_dtypes
    from concourse import bass_utils

    if TRACE:
        _ensure_ntff_hook()

    f8e4 = ml_dtypes.float8_e3m4
    bf16 = ml_dtypes.bfloat16
    x = np.asarray(x, dtype=np.float32)
    seg = np.asarray(segment_ids).astype(np.int64)
    weight = np.asarray(weight, dtype=np.float32)
    bias = np.asarray(bias, dtype=np.float32)
    G = int(num_graphs)
    assert G == NUM_GRAPHS and x.shape == (N_NODES, IN_CH)

    bounds = np.searchsorted(seg, np.arange(G + 1))
    cnts = np.diff(bounds).astype(np.int64)
    m = (cnts + 7) // 8                      # slot rows per seg

    # ---- snake-pack segments into 512 bins of exactly 32 segs ----
    order = np.argsort(-m, kind="stable")
    bin_of_seg = np.empty(G, np.int64)
    local_of_seg = np.empty(G, np.int64)
    fwd = np.arange(NBIN)
    rev = fwd[::-1]
    for r in range(G // NBIN):               # 32 rounds
        rowsegs = order[r * NBIN:(r + 1) * NBIN]
        bins = fwd if r % 2 == 0 else rev
        bin_of_seg[rowsegs] = bins
        local_of_seg[rowsegs] = r
    R = np.zeros(NBIN, np.int64)
    np.add.at(R, bin_of_seg, m)
    T = int(np.ceil(R.max() / P))
    assert T * P >= R.max()

    # per-seg starting slot row within its block (assignment order per bin)
    row_start = np.zeros(G, np.int64)
    base = np.zeros(NBIN, np.int64)
    for r in range(G // NBIN):
        rowsegs = order[r * NBIN:(r + 1) * NBIN]
        b = bin_of_seg[rowsegs]
        row_start[rowsegs] = base[b]
        base[b] += m[rowsegs]

    # ---- sigma-delta quantize on e4m3 grid ----
    L = int(m.max() * 8)
    idx_in_seg = np.arange(N_NODES) - bounds[seg]
    xpad = np.zeros((G, L, IN_CH), np.float32)
    vpad = np.zeros((G, L), bool)
    xpad[seg, idx_in_seg] = x
    vpad[seg, idx_in_seg] = True
    q = _sigma_delta_fp8(xpad, vpad, f8e4)   # [G, L, F]
    del xpad, vpad
    q = q.reshape(G, L // 8, 8, IN_CH)

    # ---- scatter into per-core slabs ----
    # per slot row: seg, row index within block
    seg_rep = np.repeat(np.arange(G), m)                    # [Rtot]
    csum = np.concatenate([[0], np.cumsum(m)])
    r_in_seg = np.arange(len(seg_rep)) - csum[seg_rep]      # [Rtot]
    row_blk = row_start[seg_rep] + r_in_seg                 # block row
    bin_r = bin_of_seg[seg_rep]
    core_r = bin_r // NBLK
    blk_r = bin_r % NBLK
    g_r = row_blk // P
    p_r = row_blk % P

    xq_all = np.zeros((N_CORES, P, NBLK * T * 512), f8e4)
    vals = q[seg_rep, r_in_seg]                             # [Rtot, 8, F]
    cols = (blk_r * (T * 512) + g_r * 512)[:, None, None] + \
        (np.arange(8) * IN_CH)[None, :, None] + \
        np.arange(IN_CH)[None, None, :]
    xq_all[core_r[:, None, None], p_r[:, None, None], cols] = vals
    del q, vals, cols

    srel_all = np.full((N_CORES, P, NBLK * T), -1.0, np.float32)
    srel_all[core_r, p_r, blk_r * T + g_r] = local_of_seg[seg_rep]

    iota = np.tile(np.arange(32, dtype=np.float32), T)[None].repeat(P, 0)

    # ---- epilogue scale + output permutation ----
    # device row (core, grp*128 + p) holds seg with bin=core*NBLK+grp*4+p//32,
    # local=p%32
    grp = np.arange(SEGS_PER_CORE) // P
    p_of = np.arange(SEGS_PER_CORE) % P
    seg_at = np.empty((N_CORES, SEGS_PER_CORE), np.int64)
    inv = np.empty(G, np.int64)
    inv[bin_of_seg * 32 + local_of_seg] = np.arange(G)
    for c in range(N_CORES):
        bins_ = c * NBLK + grp * 4 + p_of // 32
        seg_at[c] = inv[bins_ * 32 + p_of % 32]
    sc = 1.0 / np.maximum(cnts, 1).astype(np.float32)[seg_at]  # [C, 2048]
    scale_all = np.ascontiguousarray(
        sc.reshape(N_CORES, NGROUP, P).transpose(0, 2, 1)).astype(np.float32)

    wb = np.concatenate([weight.T, bias[None]], axis=0).astype(np.float16)
    ident = np.eye(P, dtype=np.float16)

    if T not in _BUILD_CACHE:
        _BUILD_CACHE[T] = _build(T)
    nc = _BUILD_CACHE[T]

    in_maps = [
        dict(xq=xq_all[c], srel=srel_all[c].astype(bf16),
             iota=iota.astype(bf16), wb=wb, ident=ident,
             scale=scale_all[c])
        for c in range(N_CORES)
    ]
    res = bass_utils.run_bass_kernel_spmd(
        nc, in_maps, core_ids=list(range(N_CORES)), trace=TRACE)
    LAST_RESULT = res
    dev = np.concatenate(
        [res.results[c]["out"] for c in range(N_CORES)], axis=0)
    out_full = np.empty((G, OUT_CH), np.float32)
    out_full[seg_at.reshape(-1)] = dev.astype(np.float32)
    return out_full
